# revision 14
# baseline (speedup 1.0000x reference)
"""Trainium2 kernel for nn_Circuit_41936060678727.

The reference is a 10-qubit real-amplitude circuit (CNOT ladders + RY
rotations) applied to an amplitude-embedded batch, measured with PauliZ on
each of the 10 wires.  Every gate is linear in the state, so the whole
8-layer circuit collapses to one fixed 784x1024 matrix W (orthonormal rows)
that depends only on `params`:

    out[b, p] = sum_z (x[b] @ W)[z]^2 * sign_p(z) / sum_z (x[b] @ W)[z]^2

The division makes the pipeline scale-invariant in y = x @ W, which lets the
matmul run in fp8 with generous global scales (SW on W, SX on centered x)
that keep everything out of e4m3's subnormal range.

Device math per core (2048 samples, data-parallel over 8 cores):
    mm1: y^T [1024, 2048] = Waug^T @ xaug         fp8 DoubleRow (0.5 cyc/row)
    sq    = y^2                                    scalar+vector engines, bf16
    mm2: o^T [11, 2048]  = Zsigns^T @ sq           bf16 (1 cyc/row)
Host: out = (o[:10] / o[10])^T, concat cores.

fp8 precision recovery (sim rel err ~1.3e-2 vs 2e-2 gate):
  - x is centered (x - 0.5) so its fp8 error halves; the constant shift is
    restored by 4 "bias rows" (ones on the x side, an fp8 split of
    0.5*colsum(W) on the W side) folded into the contraction for free.
  - 784 main rows pad to 5 DoubleRow chunks of 256 rows; the 496 spare
    slots carry residual-correction rows (W - fp8(W) paired with the same
    x values) that cancel most of the W quantization error.  Chunk 3 is
    the rows-0:256 residual at e5m2 (wide exponent range) and reuses the
    chunk-0 x tile already in SBUF.
"""

import numpy as np
import ml_dtypes

N_QUBITS = 10
DIM = 1 << N_QUBITS          # 1024
N_OUT = 10
D_IN = 784
B_TOTAL = 16384
N_CORES = 8
B_CORE = B_TOTAL // N_CORES  # 2048
GROUP = 512                  # batch columns per matmul (one PSUM bank, fp32)
N_GROUPS = B_CORE // GROUP   # 4
Z_CHUNK = 128
N_ZCH = DIM // Z_CHUNK       # 8
ZCOLS = 16                   # cols 0..9 = PauliZ signs, 10 = ones, 11..15 pad
NCH = 4                      # DoubleRow K-chunks of 256 rows (1024 slots)
SW = 64.0                    # global W scale (pulls W out of e4m3 subnormals)
SX = 4.0                     # global centered-x scale
N_WARM = 18                  # PE warm-up matmuls (clock ramp + DMA prefix)

E4 = ml_dtypes.float8_e4m3
E5 = ml_dtypes.float8_e5m2


# ----------------------------------------------------------------------------
# Host-side precompute: collapse the circuit to W = M[:784, :]
# ----------------------------------------------------------------------------

def _apply_ry(S, theta, q):
    B = S.shape[0]
    left, right = 1 << q, 1 << (N_QUBITS - q - 1)
    s = S.reshape(B, left, 2, right)
    c, sn = np.cos(theta / 2), np.sin(theta / 2)
    s0 = c * s[:, :, 0] - sn * s[:, :, 1]
    s1 = sn * s[:, :, 0] + c * s[:, :, 1]
    return np.stack([s0, s1], axis=2).reshape(B, DIM)


def _apply_cnot(S, q):
    B = S.shape[0]
    left, right = 1 << q, 1 << (N_QUBITS - q - 2)
    s = S.reshape(B, left, 2, 2, right)
    s = np.concatenate([s[:, :, :1], np.flip(s[:, :, 1:], axis=3)], axis=2)
    return s.reshape(B, DIM)


def _build_W(params):
    """Circuit applied to basis rows e_0..e_783 -> W[784, 1024], fp64."""
    w = np.pi * np.tanh(params.astype(np.float64))
    S = np.zeros((D_IN, DIM), dtype=np.float64)
    S[np.arange(D_IN), np.arange(D_IN)] = 1.0
    for l in range(params.shape[0]):
        for start in (0, 1):
            for i in range(start, N_QUBITS - 1, 2):
                S = _apply_cnot(S, i)
        for i in range(N_QUBITS):
            S = _apply_ry(S, w[l, i], i)
    return S


def _build_Z():
    z = np.arange(DIM)
    Z = np.zeros((DIM, ZCOLS), dtype=np.float32)
    for p in range(N_OUT):
        Z[:, p] = 1.0 - 2.0 * ((z >> (N_QUBITS - 1 - p)) & 1)
    Z[:, N_OUT] = 1.0
    # device layout [128, 8*16]: z-chunk c rows c*128..c*128+128 at cols c*16..
    Zd = Z.reshape(N_ZCH, Z_CHUNK, ZCOLS).transpose(1, 0, 2).reshape(Z_CHUNK, -1)
    return np.ascontiguousarray(Zd)


def _q(a, t):
    return np.asarray(a, np.float32).astype(t)


def _chunk_to_tile(A):
    """[256 aug rows, n] -> [128 partitions, 2 halves, n]; slot (p, i) holds
    aug row i*128 + p.  Must match between the W and x sides (it does)."""
    n = A.shape[1]
    return np.ascontiguousarray(A.reshape(2, 128, n).transpose(1, 0, 2))


def _build_weight_operands(params):
    """Returns w4 [8, 128, 4, 256] e4m3."""
    W = _build_W(params)                     # fp64 [784, 1024]
    Ws = W * SW
    Wh = _q(Ws, E4)                          # main fp8 weights
    Wl = Ws - Wh.astype(np.float64)          # residual
    c_s = 0.5 * W.sum(axis=0) * SW * SX      # centering bias, scaled domain
    bias = []
    r = c_s.copy()
    b = _q(r / 2, E4); bias.append(b); r -= b.astype(np.float64)
    for _ in range(3):
        b = _q(r, E4); bias.append(b); r -= b.astype(np.float64)

    # e4m3 chunks in processing order [rows 0:256, 256:512, 512:768, mixed]
    che4 = [
        Wh[0:256], Wh[256:512], Wh[512:768],
        np.concatenate([
            _q(Ws[768:784], E4),             # main tail rows 768..783
            np.stack(bias, axis=0),          # 4 bias rows (x side = ones)
            _q(Wl[0:236], E4),               # residual rows 0..235
        ], axis=0),
    ]

    w4 = np.empty((N_ZCH, 128, 4, 2 * Z_CHUNK), dtype=E4)
    for j, A in enumerate(che4):
        T = _chunk_to_tile(np.ascontiguousarray(A))      # [128, 2, 1024]
        for z in range(N_ZCH):
            blk = T[:, :, z * Z_CHUNK:(z + 1) * Z_CHUNK]  # [128, 2, 128]
            w4[z, :, j, :] = blk.reshape(128, 2 * Z_CHUNK)
    return w4


def _build_x_operand(x_core):
    """x [2048, 784] f32 -> xt [4, 128, 2, 2048] e4m3 (aug chunks 0-2 main,
    chunk 3 = [tail rows 768:784, ones x4, rows 256:492])."""
    xs = (x_core.astype(np.float64) - 0.5) * SX
    xh = _q(xs, E4)                          # [2048, 784]
    xT = np.ascontiguousarray(xh.T)          # [784, 2048]
    ones = np.ones((4, B_CORE), dtype=E4)
    chunks = [
        xT[0:256], xT[256:512], xT[512:768],
        np.concatenate([xT[768:784], ones, xT[0:236]], axis=0),
    ]
    xt = np.empty((4, 128, 2, B_CORE), dtype=E4)
    for c, A in enumerate(chunks):
        xt[c] = _chunk_to_tile(np.ascontiguousarray(A))
    return xt


def _round_f32r(a):
    """fp32 -> float32r encoding (e8m11, RNE): low 12 mantissa bits cleared."""
    u = np.ascontiguousarray(a, dtype=np.float32).view(np.uint32)
    keep = u & np.uint32(0xFFFFF000)
    rem = u & np.uint32(0xFFF)
    inc = (rem > 0x800) | ((rem == 0x800) & (((u >> 12) & 1) == 1))
    out = keep + (inc.astype(np.uint32) << 12)
    return out.view(np.float32)


# ----------------------------------------------------------------------------
# Bass program (identical SPMD program on all 8 cores)
# ----------------------------------------------------------------------------

_NC_CACHE = {}
TRACE = False           # test harness can flip this for profiling
LAST_RESULTS = None


def _build_bass():
    from contextlib import ExitStack

    import concourse.tile as tile
    from concourse import bacc, mybir

    f32 = mybir.dt.float32
    f32r = mybir.dt.float32r
    f8e4 = mybir.dt.float8e4
    f8e5 = mybir.dt.float8e5
    bf16 = mybir.dt.bfloat16
    DR = mybir.MatmulPerfMode.DoubleRow
    MULT = mybir.AluOpType.mult

    nc = bacc.Bacc(
        "TRN2", target_bir_lowering=False, debug=False, num_devices=N_CORES
    )
    xt_d = nc.declare_dram_parameter("xt", [4, 128, 2, B_CORE], f8e4, isOutput=False)
    w4_d = nc.declare_dram_parameter("w4", [N_ZCH, 128, 4, 256], f8e4, isOutput=False)
    zt_d = nc.declare_dram_parameter("zt", [Z_CHUNK, N_ZCH * ZCOLS], f32r, isOutput=False)
    out_d = nc.declare_dram_parameter("out", [N_OUT + 1, B_CORE], f32, isOutput=True)

    N_PH = 4                      # column phases of one 512-col group each

    with ExitStack() as ctx:
        tc = ctx.enter_context(tile.TileContext(nc))
        wpool = ctx.enter_context(tc.tile_pool(name="w", bufs=1))
        xpool = ctx.enter_context(tc.tile_pool(name="x", bufs=1))
        zpool = ctx.enter_context(tc.tile_pool(name="z", bufs=1))
        sqpool = ctx.enter_context(tc.tile_pool(name="sq", bufs=3))
        opool = ctx.enter_context(tc.tile_pool(name="osb", bufs=1))
        pypool = ctx.enter_context(tc.tile_pool(name="py", bufs=3, space="PSUM"))
        popool = ctx.enter_context(tc.tile_pool(name="po", bufs=1, space="PSUM"))

        # PE pre-warm: K=128 bf16 matmuls with the array fully lit -- the
        # HAM/DVFS clock ramps on real array activity (K=1 warms leave it at
        # 1.2 GHz), and the warm window also covers the first x slab's DMA.
        warm_in = opool.tile([128, 384], bf16, name="warm_in")
        nc.any.memset(warm_in[:], 1.0)
        warm_ps = popool.tile([128, 256], f32, name="warm_ps", tag="warm")
        for _ in range(N_WARM):
            nc.tensor.matmul(
                warm_ps[:],
                lhsT=warm_in[:, 0:128],
                rhs=warm_in[:, 128:384],
                start=True,
                stop=True,
                skip_group_check=True,
            )

        # Input DMAs, split over two rings (sync + gpsimd) so the phase-0
        # critical set (signs, z0 weights, first x quarter) streams first.
        z_sb = zpool.tile([Z_CHUNK, N_ZCH * ZCOLS], f32r)
        nc.sync.dma_start(z_sb[:], zt_d[:, :])
        w4_sb = [None] * N_ZCH
        x_sb = [[None] * N_PH for _ in range(4)]

        def load_w(z, eng=None):
            t = wpool.tile([128, 4, 2, Z_CHUNK], f8e4, tag=f"w4_{z}", name=f"w4_{z}")
            (eng or nc.sync).dma_start(t[:], w4_d[z, :, :, :])
            w4_sb[z] = t

        def load_x(c, q, eng):
            t = xpool.tile([128, 2, GROUP], f8e4, tag=f"x{c}q{q}", name=f"x{c}q{q}")
            eng.dma_start(t[:], xt_d[c, :, :, q * GROUP:(q + 1) * GROUP])
            x_sb[c][q] = t

        load_w(0)
        load_x(0, 0, nc.sync)
        load_x(1, 0, nc.sync)
        load_x(2, 0, nc.gpsimd)
        load_x(3, 0, nc.gpsimd)
        load_w(1, nc.gpsimd)
        load_w(2)
        load_w(3, nc.gpsimd)
        load_w(4)
        load_x(0, 1, nc.sync)
        load_x(1, 1, nc.sync)
        load_x(2, 1, nc.gpsimd)
        load_x(3, 1, nc.gpsimd)
        load_w(5, nc.gpsimd)
        load_w(6)
        load_w(7, nc.gpsimd)
        load_x(0, 2, nc.sync)
        load_x(1, 2, nc.sync)
        load_x(2, 2, nc.gpsimd)
        load_x(3, 2, nc.gpsimd)
        load_x(0, 3, nc.sync)
        load_x(1, 3, nc.sync)
        load_x(2, 3, nc.gpsimd)
        load_x(3, 3, nc.gpsimd)

        out_sb = opool.tile([N_OUT + 1, B_CORE], f32)

        def emit_mm2(po, sq, z):
            nc.tensor.matmul(
                po[:],
                lhsT=z_sb[:, z * ZCOLS:z * ZCOLS + N_OUT + 1],
                rhs=sq[:],
                start=(z == 0),
                stop=(z == N_ZCH - 1),
                skip_group_check=True,
            )

        def finish_phase(h, po):
            off = h * GROUP
            nc.vector.tensor_copy(out_sb[:, off:off + GROUP], po[:])
            nc.sync.dma_start(
                out_d[:, off:off + GROUP], out_sb[:, off:off + GROUP]
            )

        # mm1 runs two z ahead of the sign-contraction, across phase
        # boundaries, so mm2 never waits on the squares' engine latency.
        pending = []        # (po, sq, z, phase) awaiting their mm2
        pos = [None] * N_PH
        for h in range(N_PH):
            pos[h] = popool.tile(
                [N_OUT + 1, GROUP], f32, tag=f"po{h % 2}", name=f"po_{h}"
            )
            for z in range(N_ZCH):
                py = pypool.tile(
                    [Z_CHUNK, GROUP], f32, tag="py", name=f"py_{h}_{z}"
                )
                for ci in range(4):
                    nc.tensor.matmul(
                        py[:],
                        lhsT=w4_sb[z][:, ci],
                        rhs=x_sb[ci][h][:],
                        start=(ci == 0),
                        stop=(ci == NCH - 1),
                        perf_mode=DR,
                        skip_group_check=True,
                    )
                if len(pending) >= 2:
                    bpo, bsq, bz, bh = pending.pop(0)
                    emit_mm2(bpo, bsq, bz)
                    if bz == N_ZCH - 1:
                        finish_phase(bh, bpo)
                sq = sqpool.tile(
                    [Z_CHUNK, GROUP], f32r, tag="sq", name=f"sq_{h}_{z}"
                )
                nc.scalar.square(sq[:], py[:])
                pending.append((pos[h], sq, z, h))
        for bpo, bsq, bz, bh in pending:
            emit_mm2(bpo, bsq, bz)
            if bz == N_ZCH - 1:
                finish_phase(bh, bpo)

    nc.finalize()
    return nc


def _get_nc():
    if "nc" not in _NC_CACHE:
        _NC_CACHE["nc"] = _build_bass()
    return _NC_CACHE["nc"]


# ----------------------------------------------------------------------------
# Entry point
# ----------------------------------------------------------------------------

def kernel(input, params):
    global LAST_RESULTS
    from concourse.bass_utils import run_bass_kernel_spmd

    x = np.ascontiguousarray(np.asarray(input, dtype=np.float32))
    p = np.asarray(params, dtype=np.float32)

    w4 = _build_weight_operands(p)
    Z = _round_f32r(_build_Z())

    nc = _get_nc()
    in_maps = []
    for c in range(N_CORES):
        xt = _build_x_operand(x[c * B_CORE:(c + 1) * B_CORE])
        in_maps.append({"xt": xt, "w4": w4, "zt": Z})

    res = run_bass_kernel_spmd(nc, in_maps, list(range(N_CORES)), trace=TRACE)
    LAST_RESULTS = res

    outs = []
    for c in range(N_CORES):
        o = res.results[c]["out"]                 # [11, 2048]
        outs.append((o[:N_OUT] / o[N_OUT:N_OUT + 1]).T)
    return np.ascontiguousarray(np.concatenate(outs, axis=0).astype(np.float32))


# revision 15
# speedup vs baseline: 1.0001x; 1.0001x over previous
"""Trainium2 kernel for nn_Circuit_41936060678727.

The reference is a 10-qubit real-amplitude circuit (CNOT ladders + RY
rotations) applied to an amplitude-embedded batch, measured with PauliZ on
each of the 10 wires.  Every gate is linear in the state, so the whole
8-layer circuit collapses to one fixed 784x1024 matrix W (orthonormal rows)
that depends only on `params`:

    out[b, p] = sum_z (x[b] @ W)[z]^2 * sign_p(z) / sum_z (x[b] @ W)[z]^2

The division makes the pipeline scale-invariant in y = x @ W, which lets the
matmul run in fp8 with generous global scales (SW on W, SX on centered x)
that keep everything out of e4m3's subnormal range.

Device math per core (2048 samples, data-parallel over 8 cores):
    mm1: y^T [1024, 2048] = Waug^T @ xaug         fp8 DoubleRow (0.5 cyc/row)
    sq    = y^2                                    scalar+vector engines, bf16
    mm2: o^T [11, 2048]  = Zsigns^T @ sq           bf16 (1 cyc/row)
Host: out = (o[:10] / o[10])^T, concat cores.

fp8 precision recovery (sim rel err ~1.3e-2 vs 2e-2 gate):
  - x is centered (x - 0.5) so its fp8 error halves; the constant shift is
    restored by 4 "bias rows" (ones on the x side, an fp8 split of
    0.5*colsum(W) on the W side) folded into the contraction for free.
  - 784 main rows pad to 5 DoubleRow chunks of 256 rows; the 496 spare
    slots carry residual-correction rows (W - fp8(W) paired with the same
    x values) that cancel most of the W quantization error.  Chunk 3 is
    the rows-0:256 residual at e5m2 (wide exponent range) and reuses the
    chunk-0 x tile already in SBUF.
"""

import numpy as np
import ml_dtypes

N_QUBITS = 10
DIM = 1 << N_QUBITS          # 1024
N_OUT = 10
D_IN = 784
B_TOTAL = 16384
N_CORES = 8
B_CORE = B_TOTAL // N_CORES  # 2048
GROUP = 512                  # batch columns per matmul (one PSUM bank, fp32)
N_GROUPS = B_CORE // GROUP   # 4
Z_CHUNK = 128
N_ZCH = DIM // Z_CHUNK       # 8
ZCOLS = 16                   # cols 0..9 = PauliZ signs, 10 = ones, 11..15 pad
NCH = 4                      # DoubleRow K-chunks of 256 rows (1024 slots)
SW = 64.0                    # global W scale (pulls W out of e4m3 subnormals)
SX = 4.0                     # global centered-x scale
N_WARM = 26                  # PE warm-up matmuls (clock ramp + DMA prefix)

E4 = ml_dtypes.float8_e4m3
E5 = ml_dtypes.float8_e5m2


# ----------------------------------------------------------------------------
# Host-side precompute: collapse the circuit to W = M[:784, :]
# ----------------------------------------------------------------------------

def _apply_ry(S, theta, q):
    B = S.shape[0]
    left, right = 1 << q, 1 << (N_QUBITS - q - 1)
    s = S.reshape(B, left, 2, right)
    c, sn = np.cos(theta / 2), np.sin(theta / 2)
    s0 = c * s[:, :, 0] - sn * s[:, :, 1]
    s1 = sn * s[:, :, 0] + c * s[:, :, 1]
    return np.stack([s0, s1], axis=2).reshape(B, DIM)


def _apply_cnot(S, q):
    B = S.shape[0]
    left, right = 1 << q, 1 << (N_QUBITS - q - 2)
    s = S.reshape(B, left, 2, 2, right)
    s = np.concatenate([s[:, :, :1], np.flip(s[:, :, 1:], axis=3)], axis=2)
    return s.reshape(B, DIM)


def _build_W(params):
    """Circuit applied to basis rows e_0..e_783 -> W[784, 1024], fp64."""
    w = np.pi * np.tanh(params.astype(np.float64))
    S = np.zeros((D_IN, DIM), dtype=np.float64)
    S[np.arange(D_IN), np.arange(D_IN)] = 1.0
    for l in range(params.shape[0]):
        for start in (0, 1):
            for i in range(start, N_QUBITS - 1, 2):
                S = _apply_cnot(S, i)
        for i in range(N_QUBITS):
            S = _apply_ry(S, w[l, i], i)
    return S


def _build_Z():
    z = np.arange(DIM)
    Z = np.zeros((DIM, ZCOLS), dtype=np.float32)
    for p in range(N_OUT):
        Z[:, p] = 1.0 - 2.0 * ((z >> (N_QUBITS - 1 - p)) & 1)
    Z[:, N_OUT] = 1.0
    # device layout [128, 8*16]: z-chunk c rows c*128..c*128+128 at cols c*16..
    Zd = Z.reshape(N_ZCH, Z_CHUNK, ZCOLS).transpose(1, 0, 2).reshape(Z_CHUNK, -1)
    return np.ascontiguousarray(Zd)


def _q(a, t):
    return np.asarray(a, np.float32).astype(t)


def _chunk_to_tile(A):
    """[256 aug rows, n] -> [128 partitions, 2 halves, n]; slot (p, i) holds
    aug row i*128 + p.  Must match between the W and x sides (it does)."""
    n = A.shape[1]
    return np.ascontiguousarray(A.reshape(2, 128, n).transpose(1, 0, 2))


def _build_weight_operands(params):
    """Returns w4 [8, 128, 4, 256] e4m3."""
    W = _build_W(params)                     # fp64 [784, 1024]
    Ws = W * SW
    Wh = _q(Ws, E4)                          # main fp8 weights
    Wl = Ws - Wh.astype(np.float64)          # residual
    c_s = 0.5 * W.sum(axis=0) * SW * SX      # centering bias, scaled domain
    bias = []
    r = c_s.copy()
    b = _q(r / 2, E4); bias.append(b); r -= b.astype(np.float64)
    for _ in range(3):
        b = _q(r, E4); bias.append(b); r -= b.astype(np.float64)

    # e4m3 chunks in processing order [rows 0:256, 256:512, 512:768, mixed]
    che4 = [
        Wh[0:256], Wh[256:512], Wh[512:768],
        np.concatenate([
            _q(Ws[768:784], E4),             # main tail rows 768..783
            np.stack(bias, axis=0),          # 4 bias rows (x side = ones)
            _q(Wl[0:236], E4),               # residual rows 0..235
        ], axis=0),
    ]

    w4 = np.empty((N_ZCH, 128, 4, 2 * Z_CHUNK), dtype=E4)
    for j, A in enumerate(che4):
        T = _chunk_to_tile(np.ascontiguousarray(A))      # [128, 2, 1024]
        for z in range(N_ZCH):
            blk = T[:, :, z * Z_CHUNK:(z + 1) * Z_CHUNK]  # [128, 2, 128]
            w4[z, :, j, :] = blk.reshape(128, 2 * Z_CHUNK)
    return w4


def _build_x_operand(x_core):
    """x [2048, 784] f32 -> xt [4, 128, 2, 2048] e4m3 (aug chunks 0-2 main,
    chunk 3 = [tail rows 768:784, ones x4, rows 256:492])."""
    xs = (x_core.astype(np.float64) - 0.5) * SX
    xh = _q(xs, E4)                          # [2048, 784]
    xT = np.ascontiguousarray(xh.T)          # [784, 2048]
    ones = np.ones((4, B_CORE), dtype=E4)
    chunks = [
        xT[0:256], xT[256:512], xT[512:768],
        np.concatenate([xT[768:784], ones, xT[0:236]], axis=0),
    ]
    xt = np.empty((4, 128, 2, B_CORE), dtype=E4)
    for c, A in enumerate(chunks):
        xt[c] = _chunk_to_tile(np.ascontiguousarray(A))
    return xt


def _round_f32r(a):
    """fp32 -> float32r encoding (e8m11, RNE): low 12 mantissa bits cleared."""
    u = np.ascontiguousarray(a, dtype=np.float32).view(np.uint32)
    keep = u & np.uint32(0xFFFFF000)
    rem = u & np.uint32(0xFFF)
    inc = (rem > 0x800) | ((rem == 0x800) & (((u >> 12) & 1) == 1))
    out = keep + (inc.astype(np.uint32) << 12)
    return out.view(np.float32)


# ----------------------------------------------------------------------------
# Bass program (identical SPMD program on all 8 cores)
# ----------------------------------------------------------------------------

_NC_CACHE = {}
TRACE = False           # test harness can flip this for profiling
LAST_RESULTS = None


def _build_bass():
    from contextlib import ExitStack

    import concourse.tile as tile
    from concourse import bacc, mybir

    f32 = mybir.dt.float32
    f32r = mybir.dt.float32r
    f8e4 = mybir.dt.float8e4
    f8e5 = mybir.dt.float8e5
    bf16 = mybir.dt.bfloat16
    DR = mybir.MatmulPerfMode.DoubleRow
    MULT = mybir.AluOpType.mult

    nc = bacc.Bacc(
        "TRN2", target_bir_lowering=False, debug=False, num_devices=N_CORES
    )
    xt_d = nc.declare_dram_parameter("xt", [4, 128, 2, B_CORE], f8e4, isOutput=False)
    w4_d = nc.declare_dram_parameter("w4", [N_ZCH, 128, 4, 256], f8e4, isOutput=False)
    zt_d = nc.declare_dram_parameter("zt", [Z_CHUNK, N_ZCH * ZCOLS], f32r, isOutput=False)
    out_d = nc.declare_dram_parameter("out", [N_OUT + 1, B_CORE], f32, isOutput=True)

    N_PH = 4                      # column phases of one 512-col group each

    with ExitStack() as ctx:
        tc = ctx.enter_context(tile.TileContext(nc))
        wpool = ctx.enter_context(tc.tile_pool(name="w", bufs=1))
        xpool = ctx.enter_context(tc.tile_pool(name="x", bufs=1))
        zpool = ctx.enter_context(tc.tile_pool(name="z", bufs=1))
        sqpool = ctx.enter_context(tc.tile_pool(name="sq", bufs=3))
        opool = ctx.enter_context(tc.tile_pool(name="osb", bufs=1))
        pypool = ctx.enter_context(tc.tile_pool(name="py", bufs=3, space="PSUM"))
        popool = ctx.enter_context(tc.tile_pool(name="po", bufs=1, space="PSUM"))

        # PE pre-warm: K=128 bf16 matmuls with the array fully lit -- the
        # HAM/DVFS clock ramps on real array activity (K=1 warms leave it at
        # 1.2 GHz), and the warm window also covers the first x slab's DMA.
        warm_in = opool.tile([128, 384], bf16, name="warm_in")
        nc.any.memset(warm_in[:], 1.0)
        warm_ps = popool.tile([128, 256], f32, name="warm_ps", tag="warm")
        for _ in range(N_WARM):
            nc.tensor.matmul(
                warm_ps[:],
                lhsT=warm_in[:, 0:128],
                rhs=warm_in[:, 128:384],
                start=True,
                stop=True,
                skip_group_check=True,
            )

        # Input DMAs, split over two rings (sync + gpsimd) so the phase-0
        # critical set (signs, z0 weights, first x quarter) streams first.
        z_sb = zpool.tile([Z_CHUNK, N_ZCH * ZCOLS], f32r)
        nc.sync.dma_start(z_sb[:], zt_d[:, :])
        w4_sb = [None] * N_ZCH
        x_sb = [[None] * N_PH for _ in range(4)]

        def load_w(z, eng=None):
            t = wpool.tile([128, 4, 2, Z_CHUNK], f8e4, tag=f"w4_{z}", name=f"w4_{z}")
            (eng or nc.sync).dma_start(t[:], w4_d[z, :, :, :])
            w4_sb[z] = t

        def load_x(c, q, eng):
            t = xpool.tile([128, 2, GROUP], f8e4, tag=f"x{c}q{q}", name=f"x{c}q{q}")
            eng.dma_start(t[:], xt_d[c, :, :, q * GROUP:(q + 1) * GROUP])
            x_sb[c][q] = t

        load_w(0)
        load_x(0, 0, nc.sync)
        load_w(1)
        load_x(0, 1, nc.sync)
        load_w(2)
        load_w(3)
        load_x(0, 2, nc.sync)
        load_w(4)
        load_w(5)
        load_x(0, 3, nc.sync)
        load_w(6)
        load_w(7)
        for q in range(N_PH):
            for c in (1, 2, 3):
                load_x(c, q, nc.gpsimd)

        out_sb = opool.tile([N_OUT + 1, B_CORE], f32)

        def emit_mm2(po, sq, z):
            nc.tensor.matmul(
                po[:],
                lhsT=z_sb[:, z * ZCOLS:z * ZCOLS + N_OUT + 1],
                rhs=sq[:],
                start=(z == 0),
                stop=(z == N_ZCH - 1),
                skip_group_check=True,
            )

        def finish_phase(h, po):
            off = h * GROUP
            nc.vector.tensor_copy(out_sb[:, off:off + GROUP], po[:])
            nc.sync.dma_start(
                out_d[:, off:off + GROUP], out_sb[:, off:off + GROUP]
            )

        # mm1 runs two z ahead of the sign-contraction, across phase
        # boundaries, so mm2 never waits on the squares' engine latency.
        pending = []        # (po, sq, z, phase) awaiting their mm2
        pos = [None] * N_PH
        for h in range(N_PH):
            pos[h] = popool.tile(
                [N_OUT + 1, GROUP], f32, tag=f"po{h % 2}", name=f"po_{h}"
            )
            for z in range(N_ZCH):
                py = pypool.tile(
                    [Z_CHUNK, GROUP], f32, tag="py", name=f"py_{h}_{z}"
                )
                for ci in range(4):
                    nc.tensor.matmul(
                        py[:],
                        lhsT=w4_sb[z][:, ci],
                        rhs=x_sb[ci][h][:],
                        start=(ci == 0),
                        stop=(ci == NCH - 1),
                        perf_mode=DR,
                        skip_group_check=True,
                    )
                if len(pending) >= 2:
                    bpo, bsq, bz, bh = pending.pop(0)
                    emit_mm2(bpo, bsq, bz)
                    if bz == N_ZCH - 1:
                        finish_phase(bh, bpo)
                sq = sqpool.tile(
                    [Z_CHUNK, GROUP], f32r, tag="sq", name=f"sq_{h}_{z}"
                )
                nc.scalar.square(sq[:], py[:])
                pending.append((pos[h], sq, z, h))
        for bpo, bsq, bz, bh in pending:
            emit_mm2(bpo, bsq, bz)
            if bz == N_ZCH - 1:
                finish_phase(bh, bpo)

    nc.finalize()
    return nc


def _get_nc():
    if "nc" not in _NC_CACHE:
        _NC_CACHE["nc"] = _build_bass()
    return _NC_CACHE["nc"]


# ----------------------------------------------------------------------------
# Entry point
# ----------------------------------------------------------------------------

def kernel(input, params):
    global LAST_RESULTS
    from concourse.bass_utils import run_bass_kernel_spmd

    x = np.ascontiguousarray(np.asarray(input, dtype=np.float32))
    p = np.asarray(params, dtype=np.float32)

    w4 = _build_weight_operands(p)
    Z = _round_f32r(_build_Z())

    nc = _get_nc()
    in_maps = []
    for c in range(N_CORES):
        xt = _build_x_operand(x[c * B_CORE:(c + 1) * B_CORE])
        in_maps.append({"xt": xt, "w4": w4, "zt": Z})

    res = run_bass_kernel_spmd(nc, in_maps, list(range(N_CORES)), trace=TRACE)
    LAST_RESULTS = res

    outs = []
    for c in range(N_CORES):
        o = res.results[c]["out"]                 # [11, 2048]
        outs.append((o[:N_OUT] / o[N_OUT:N_OUT + 1]).T)
    return np.ascontiguousarray(np.concatenate(outs, axis=0).astype(np.float32))


# revision 16
# speedup vs baseline: 1.0121x; 1.0120x over previous
"""Trainium2 kernel for nn_Circuit_41936060678727.

The reference is a 10-qubit real-amplitude circuit (CNOT ladders + RY
rotations) applied to an amplitude-embedded batch, measured with PauliZ on
each of the 10 wires.  Every gate is linear in the state, so the whole
8-layer circuit collapses to one fixed 784x1024 matrix W (orthonormal rows)
that depends only on `params`:

    out[b, p] = sum_z (x[b] @ W)[z]^2 * sign_p(z) / sum_z (x[b] @ W)[z]^2

The division makes the pipeline scale-invariant in y = x @ W, which lets the
matmul run in fp8 with generous global scales (SW on W, SX on centered x)
that keep everything out of e4m3's subnormal range.

Device math per core (2048 samples, data-parallel over 8 cores):
    mm1: y^T [1024, 2048] = Waug^T @ xaug         fp8 DoubleRow (0.5 cyc/row)
    sq    = y^2                                    scalar+vector engines, bf16
    mm2: o^T [11, 2048]  = Zsigns^T @ sq           bf16 (1 cyc/row)
Host: out = (o[:10] / o[10])^T, concat cores.

fp8 precision recovery (sim rel err ~1.3e-2 vs 2e-2 gate):
  - x is centered (x - 0.5) so its fp8 error halves; the constant shift is
    restored by 4 "bias rows" (ones on the x side, an fp8 split of
    0.5*colsum(W) on the W side) folded into the contraction for free.
  - 784 main rows pad to 5 DoubleRow chunks of 256 rows; the 496 spare
    slots carry residual-correction rows (W - fp8(W) paired with the same
    x values) that cancel most of the W quantization error.  Chunk 3 is
    the rows-0:256 residual at e5m2 (wide exponent range) and reuses the
    chunk-0 x tile already in SBUF.
"""

import numpy as np
import ml_dtypes

N_QUBITS = 10
DIM = 1 << N_QUBITS          # 1024
N_OUT = 10
D_IN = 784
B_TOTAL = 16384
N_CORES = 8
B_CORE = B_TOTAL // N_CORES  # 2048
GROUP = 512                  # batch columns per matmul (one PSUM bank, fp32)
N_GROUPS = B_CORE // GROUP   # 4
Z_CHUNK = 128
N_ZCH = DIM // Z_CHUNK       # 8
ZCOLS = 16                   # cols 0..9 = PauliZ signs, 10 = ones, 11..15 pad
NCH = 4                      # DoubleRow K-chunks of 256 rows (1024 slots)
SW = 64.0                    # global W scale (pulls W out of e4m3 subnormals)
SX = 4.0                     # global centered-x scale
N_WARM = 22                  # PE warm-up matmuls (clock ramp + DMA prefix)

E4 = ml_dtypes.float8_e4m3
E5 = ml_dtypes.float8_e5m2


# ----------------------------------------------------------------------------
# Host-side precompute: collapse the circuit to W = M[:784, :]
# ----------------------------------------------------------------------------

def _apply_ry(S, theta, q):
    B = S.shape[0]
    left, right = 1 << q, 1 << (N_QUBITS - q - 1)
    s = S.reshape(B, left, 2, right)
    c, sn = np.cos(theta / 2), np.sin(theta / 2)
    s0 = c * s[:, :, 0] - sn * s[:, :, 1]
    s1 = sn * s[:, :, 0] + c * s[:, :, 1]
    return np.stack([s0, s1], axis=2).reshape(B, DIM)


def _apply_cnot(S, q):
    B = S.shape[0]
    left, right = 1 << q, 1 << (N_QUBITS - q - 2)
    s = S.reshape(B, left, 2, 2, right)
    s = np.concatenate([s[:, :, :1], np.flip(s[:, :, 1:], axis=3)], axis=2)
    return s.reshape(B, DIM)


def _build_W(params):
    """Circuit applied to basis rows e_0..e_783 -> W[784, 1024], fp64."""
    w = np.pi * np.tanh(params.astype(np.float64))
    S = np.zeros((D_IN, DIM), dtype=np.float64)
    S[np.arange(D_IN), np.arange(D_IN)] = 1.0
    for l in range(params.shape[0]):
        for start in (0, 1):
            for i in range(start, N_QUBITS - 1, 2):
                S = _apply_cnot(S, i)
        for i in range(N_QUBITS):
            S = _apply_ry(S, w[l, i], i)
    return S


def _build_Z():
    z = np.arange(DIM)
    Z = np.zeros((DIM, ZCOLS), dtype=np.float32)
    for p in range(N_OUT):
        Z[:, p] = 1.0 - 2.0 * ((z >> (N_QUBITS - 1 - p)) & 1)
    Z[:, N_OUT] = 1.0
    # device layout [128, 8*16]: z-chunk c rows c*128..c*128+128 at cols c*16..
    Zd = Z.reshape(N_ZCH, Z_CHUNK, ZCOLS).transpose(1, 0, 2).reshape(Z_CHUNK, -1)
    return np.ascontiguousarray(Zd)


def _q(a, t):
    return np.asarray(a, np.float32).astype(t)


def _chunk_to_tile(A):
    """[256 aug rows, n] -> [128 partitions, 2 halves, n]; slot (p, i) holds
    aug row i*128 + p.  Must match between the W and x sides (it does)."""
    n = A.shape[1]
    return np.ascontiguousarray(A.reshape(2, 128, n).transpose(1, 0, 2))


def _build_weight_operands(params):
    """Returns w4 [8, 128, 4, 256] e4m3."""
    W = _build_W(params)                     # fp64 [784, 1024]
    Ws = W * SW
    Wh = _q(Ws, E4)                          # main fp8 weights
    Wl = Ws - Wh.astype(np.float64)          # residual
    c_s = 0.5 * W.sum(axis=0) * SW * SX      # centering bias, scaled domain
    bias = []
    r = c_s.copy()
    b = _q(r / 2, E4); bias.append(b); r -= b.astype(np.float64)
    for _ in range(3):
        b = _q(r, E4); bias.append(b); r -= b.astype(np.float64)

    # e4m3 chunks in processing order [rows 0:256, 256:512, 512:768, mixed]
    che4 = [
        Wh[0:256], Wh[256:512], Wh[512:768],
        np.concatenate([
            _q(Ws[768:784], E4),             # main tail rows 768..783
            np.stack(bias, axis=0),          # 4 bias rows (x side = ones)
            _q(Wl[0:236], E4),               # residual rows 0..235
        ], axis=0),
    ]

    w4 = np.empty((N_ZCH, 128, 4, 2 * Z_CHUNK), dtype=E4)
    for j, A in enumerate(che4):
        T = _chunk_to_tile(np.ascontiguousarray(A))      # [128, 2, 1024]
        for z in range(N_ZCH):
            blk = T[:, :, z * Z_CHUNK:(z + 1) * Z_CHUNK]  # [128, 2, 128]
            w4[z, :, j, :] = blk.reshape(128, 2 * Z_CHUNK)
    return w4


def _build_x_operand(x_core):
    """x [2048, 784] f32 -> xt [4, 128, 2, 2048] e4m3 (aug chunks 0-2 main,
    chunk 3 = [tail rows 768:784, ones x4, rows 256:492])."""
    xs = (x_core.astype(np.float64) - 0.5) * SX
    xh = _q(xs, E4)                          # [2048, 784]
    xT = np.ascontiguousarray(xh.T)          # [784, 2048]
    ones = np.ones((4, B_CORE), dtype=E4)
    chunks = [
        xT[0:256], xT[256:512], xT[512:768],
        np.concatenate([xT[768:784], ones, xT[0:236]], axis=0),
    ]
    xt = np.empty((4, 128, 2, B_CORE), dtype=E4)
    for c, A in enumerate(chunks):
        xt[c] = _chunk_to_tile(np.ascontiguousarray(A))
    return xt


def _round_f32r(a):
    """fp32 -> float32r encoding (e8m11, RNE): low 12 mantissa bits cleared."""
    u = np.ascontiguousarray(a, dtype=np.float32).view(np.uint32)
    keep = u & np.uint32(0xFFFFF000)
    rem = u & np.uint32(0xFFF)
    inc = (rem > 0x800) | ((rem == 0x800) & (((u >> 12) & 1) == 1))
    out = keep + (inc.astype(np.uint32) << 12)
    return out.view(np.float32)


# ----------------------------------------------------------------------------
# Bass program (identical SPMD program on all 8 cores)
# ----------------------------------------------------------------------------

_NC_CACHE = {}
TRACE = False           # test harness can flip this for profiling
LAST_RESULTS = None


def _build_bass():
    from contextlib import ExitStack

    import concourse.tile as tile
    from concourse import bacc, mybir

    f32 = mybir.dt.float32
    f32r = mybir.dt.float32r
    f8e4 = mybir.dt.float8e4
    f8e5 = mybir.dt.float8e5
    bf16 = mybir.dt.bfloat16
    DR = mybir.MatmulPerfMode.DoubleRow
    MULT = mybir.AluOpType.mult

    nc = bacc.Bacc(
        "TRN2", target_bir_lowering=False, debug=False, num_devices=N_CORES
    )
    xt_d = nc.declare_dram_parameter("xt", [4, 128, 2, B_CORE], f8e4, isOutput=False)
    w4_d = nc.declare_dram_parameter("w4", [N_ZCH, 128, 4, 256], f8e4, isOutput=False)
    zt_d = nc.declare_dram_parameter("zt", [Z_CHUNK, N_ZCH * ZCOLS], f32r, isOutput=False)
    out_d = nc.declare_dram_parameter("out", [N_OUT + 1, B_CORE], f32, isOutput=True)

    N_PH = 4                      # column phases of one 512-col group each

    with ExitStack() as ctx:
        tc = ctx.enter_context(tile.TileContext(nc))
        wpool = ctx.enter_context(tc.tile_pool(name="w", bufs=1))
        xpool = ctx.enter_context(tc.tile_pool(name="x", bufs=1))
        zpool = ctx.enter_context(tc.tile_pool(name="z", bufs=1))
        sqpool = ctx.enter_context(tc.tile_pool(name="sq", bufs=3))
        opool = ctx.enter_context(tc.tile_pool(name="osb", bufs=1))
        pypool = ctx.enter_context(tc.tile_pool(name="py", bufs=2, space="PSUM"))
        popool = ctx.enter_context(tc.tile_pool(name="po", bufs=1, space="PSUM"))

        # PE pre-warm: K=128 bf16 matmuls with the array fully lit -- the
        # HAM/DVFS clock ramps on real array activity (K=1 warms leave it at
        # 1.2 GHz), and the warm window also covers the first x slab's DMA.
        warm_in = opool.tile([128, 384], bf16, name="warm_in")
        nc.any.memset(warm_in[:], 1.0)
        warm_ps = popool.tile([128, 256], f32, name="warm_ps", tag="warm")
        for _ in range(N_WARM):
            nc.tensor.matmul(
                warm_ps[:],
                lhsT=warm_in[:, 0:128],
                rhs=warm_in[:, 128:384],
                start=True,
                stop=True,
                skip_group_check=True,
            )

        # Input DMAs, split over two rings (sync + gpsimd) so the phase-0
        # critical set (signs, z0 weights, first x quarter) streams first.
        z_sb = zpool.tile([Z_CHUNK, N_ZCH * ZCOLS], f32r)
        nc.sync.dma_start(z_sb[:], zt_d[:, :])
        w4_sb = [None] * N_ZCH
        x_sb = [[None] * N_PH for _ in range(4)]

        def load_w(z, eng=None):
            t = wpool.tile([128, 4, 2, Z_CHUNK], f8e4, tag=f"w4_{z}", name=f"w4_{z}")
            (eng or nc.sync).dma_start(t[:], w4_d[z, :, :, :])
            w4_sb[z] = t

        def load_x(c, q, eng):
            t = xpool.tile([128, 2, GROUP], f8e4, tag=f"x{c}q{q}", name=f"x{c}q{q}")
            eng.dma_start(t[:], xt_d[c, :, :, q * GROUP:(q + 1) * GROUP])
            x_sb[c][q] = t

        load_w(0)
        load_x(0, 0, nc.sync)
        load_w(1)
        load_x(0, 1, nc.sync)
        load_w(2)
        load_w(3)
        load_x(0, 2, nc.sync)
        load_w(4)
        load_w(5)
        load_x(0, 3, nc.sync)
        load_w(6)
        load_w(7)
        for q in range(N_PH):
            for c in (1, 2, 3):
                load_x(c, q, nc.gpsimd)

        out_sb = opool.tile([N_OUT + 1, B_CORE], f32)

        for h in range(N_PH):
            po = popool.tile(
                [N_OUT + 1, GROUP], f32, tag=f"po{h % 2}", name=f"po_{h}"
            )

            def emit_mm2(sq, z):
                nc.tensor.matmul(
                    po[:],
                    lhsT=z_sb[:, z * ZCOLS:z * ZCOLS + N_OUT + 1],
                    rhs=sq[:],
                    start=(z == 0),
                    stop=(z == N_ZCH - 1),
                    skip_group_check=True,
                )

            sqs = []
            for z in range(N_ZCH):
                py = pypool.tile(
                    [Z_CHUNK, GROUP], f32, tag="py", name=f"py_{h}_{z}"
                )
                for ci in range(4):
                    nc.tensor.matmul(
                        py[:],
                        lhsT=w4_sb[z][:, ci],
                        rhs=x_sb[ci][h][:],
                        start=(ci == 0),
                        stop=(ci == NCH - 1),
                        perf_mode=DR,
                        skip_group_check=True,
                    )
                # sign-contraction runs two z behind mm1, so it never waits
                # on the square's engine latency
                if z >= 2:
                    emit_mm2(sqs[z - 2], z - 2)
                sq = sqpool.tile(
                    [Z_CHUNK, GROUP], f32r, tag="sq", name=f"sq_{h}_{z}"
                )
                nc.scalar.square(sq[:], py[:])
                sqs.append(sq)
            emit_mm2(sqs[N_ZCH - 2], N_ZCH - 2)
            emit_mm2(sqs[N_ZCH - 1], N_ZCH - 1)
            off = h * GROUP
            nc.vector.tensor_copy(out_sb[:, off:off + GROUP], po[:])
            nc.sync.dma_start(
                out_d[:, off:off + GROUP], out_sb[:, off:off + GROUP]
            )

    nc.finalize()
    return nc


def _get_nc():
    if "nc" not in _NC_CACHE:
        _NC_CACHE["nc"] = _build_bass()
    return _NC_CACHE["nc"]


# ----------------------------------------------------------------------------
# Entry point
# ----------------------------------------------------------------------------

def kernel(input, params):
    global LAST_RESULTS
    from concourse.bass_utils import run_bass_kernel_spmd

    x = np.ascontiguousarray(np.asarray(input, dtype=np.float32))
    p = np.asarray(params, dtype=np.float32)

    w4 = _build_weight_operands(p)
    Z = _round_f32r(_build_Z())

    nc = _get_nc()
    in_maps = []
    for c in range(N_CORES):
        xt = _build_x_operand(x[c * B_CORE:(c + 1) * B_CORE])
        in_maps.append({"xt": xt, "w4": w4, "zt": Z})

    res = run_bass_kernel_spmd(nc, in_maps, list(range(N_CORES)), trace=TRACE)
    LAST_RESULTS = res

    outs = []
    for c in range(N_CORES):
        o = res.results[c]["out"]                 # [11, 2048]
        outs.append((o[:N_OUT] / o[N_OUT:N_OUT + 1]).T)
    return np.ascontiguousarray(np.concatenate(outs, axis=0).astype(np.float32))
